# revision 1
# baseline (speedup 1.0000x reference)
"""Trainium2 (Bass) SPMD kernel for the CBGNN message-passing problem. v3.

See kernel.py (v2) docstring for the overall structure.  v3 adds:
  - no all-engine barriers: every internal DRAM tensor has at most two
    writers, so Tile's dependency tracking emits small per-instruction
    wait sets (the ISA has a hard cap on sync-wait commands per
    instruction);
  - all weights/constants packed into one input tensor (one DMA lane);
  - phase-A x tiles loaded with a single DMA covering both k-chunks;
  - phase-B/C scan outputs staged in SBUF and written with one DMA each;
  - phase D split: targets with at least one Edge2cycle row go through the
    full MLP (capacity 36864/core); for empty targets om==0 makes the MLP
    collapse to a closed form in whether_k (leaky(wk*B) == wk*leaky(B) for
    wk>=0), evaluated in a handful of elementwise ops.
"""

import sys

import numpy as np

for _p in ("/opt/trn_rl_repo",):
    if _p not in sys.path:
        sys.path.insert(0, _p)

NCORES = 8
P = 128


class Cfg:
    n_cyc = 262144
    out_dim = 256
    e_cc = 4194304
    m_e2c = 4194304
    len_edges = 1048576
    t1 = 4224            # phase-B stream columns per partition row
    t2 = 4224            # phase-C stream columns per partition row
    a_mac = 2048         # phase-A macro tile (tokens)
    tokd_full = 36864    # phase-D full-MLP capacity per core
    d_chunk = 9216       # phase-D featT chunk (tokens)
    gather_chunks = 8    # indirect-gather splits
    neg_slope = 0.2
    ln_eps = 1e-5
    cshift = 64.0        # positivity shift for the max-scan
    mlp_bf16 = True
    phases = "abcd"

    @property
    def seg_pc(self):
        return self.n_cyc // NCORES

    @property
    def toka(self):
        return self.n_cyc // NCORES

    @property
    def tgt_pc(self):
        return self.len_edges // NCORES

    @property
    def tokd(self):
        return self.len_edges // NCORES


class SmallCfg(Cfg):
    n_cyc = 16384
    e_cc = 65536
    m_e2c = 65536
    len_edges = 16384
    t1 = 96
    t2 = 96
    a_mac = 2048
    tokd_full = 2304
    d_chunk = 2304
    gather_chunks = 2


# ---------------------------------------------------------------------------
# host-side sharding / layout (index work + parameter folding only)
# ---------------------------------------------------------------------------

def _pack_rows(counts, ncols):
    csum = np.cumsum(counts)
    nseg = len(counts)
    seg_row = np.empty(nseg, np.int64)
    seg_col0 = np.empty(nseg, np.int64)
    start = 0
    base = 0
    for r in range(P):
        j = int(np.searchsorted(csum, base + ncols, side="right"))
        if j < nseg and counts[j] > ncols:
            raise ValueError("segment larger than a row")
        prev = base
        seg_row[start:j] = r
        seg_col0[start:j] = (csum[start:j] - counts[start:j]) - prev
        if j > 0:
            base = int(csum[j - 1])
        start = j
        if start == nseg:
            break
    if start != nseg:
        raise ValueError("edges did not fit into P rows")
    return seg_row, seg_col0


def _layout_stream(named_vals, seg_local, seg_counts, seg_starts_local, ncols):
    seg_row, seg_col0 = _pack_rows(seg_counts, ncols)
    rank = np.arange(len(seg_local)) - seg_starts_local[seg_local]
    slot = seg_row[seg_local] * ncols + seg_col0[seg_local] + rank
    out = {}
    for name, (vals, fill) in named_vals.items():
        arr = np.full(P * ncols, fill, dtype=np.asarray(vals).dtype)
        arr[slot] = vals
        out[name] = arr.reshape(P, ncols)
    lab = np.full(P * ncols, -1, np.int64)
    lab[slot] = seg_local
    lab2 = lab.reshape(P, ncols)
    msk = np.zeros((P, ncols), np.float32)
    msk[:, 1:] = ((lab2[:, 1:] == lab2[:, :-1]) & (lab2[:, 1:] >= 0)).astype(
        np.float32)
    out["__mask"] = msk
    end_slot = seg_row * ncols + seg_col0 + seg_counts - 1
    filler_slot = P * ncols - 1
    assert lab.reshape(-1)[filler_slot] == -1, "last slot is not filler"
    out["__end"] = np.where(seg_counts > 0, end_slot,
                            filler_slot).astype(np.int32)
    return out


def _fold_mlp(W1, g, b, W2, b2):
    W2 = np.asarray(W2, np.float64).reshape(-1)
    g = np.asarray(g, np.float64)
    b = np.asarray(b, np.float64)
    w2eff = g * W2
    b2eff = float(np.asarray(b2).reshape(-1)[0]) + float(np.dot(b, W2))
    return (w2eff.astype(np.float32), np.float32(b2eff),
            np.float32(w2eff.sum()))


def host_prepare(inputs, cfg):
    n_cyc, seg_pc, tgt_pc = cfg.n_cyc, cfg.seg_pc, cfg.tgt_pc
    od = cfg.out_dim

    x = np.asarray(inputs["x"], np.float32)
    e2c = np.asarray(inputs["Edge2cycle"])
    eidx = np.asarray(inputs["edge_index"])
    pce = np.asarray(inputs["permuteCE"], np.float32)
    wk = np.asarray(inputs["whether_k"], np.float32)
    assert float(wk.min()) >= 0.0, "fast path requires whether_k >= 0"

    seg0 = np.asarray(eidx[0], np.int64)
    seg1 = np.asarray(eidx[1], np.int64)

    xT = np.ascontiguousarray(x.T)
    spr = cfg.toka // P

    def tab_pos(g):
        core = g // seg_pc
        loc = g - core * seg_pc
        return core * seg_pc + (loc % P) * spr + loc // P

    order0 = np.argsort(seg0, kind="stable")
    seg0s = seg0[order0]
    cnt0 = np.bincount(seg0, minlength=n_cyc).astype(np.int64)
    start0 = np.zeros(n_cyc + 1, np.int64)
    np.cumsum(cnt0, out=start0[1:])

    tgt = np.asarray(e2c[:, 0], np.int64)
    src = np.asarray(e2c[:, 1], np.int64)
    order1 = np.argsort(tgt, kind="stable")
    tgts = tgt[order1]
    cnt1 = np.bincount(tgt, minlength=cfg.len_edges).astype(np.int64)
    start1 = np.zeros(cfg.len_edges + 1, np.int64)
    np.cumsum(cnt1, out=start1[1:])

    sent = np.int32(n_cyc)

    # ---- parameter folding
    w2eff, b2e, s2 = _fold_mlp(inputs["W1"], inputs["g1"], inputs["b1"],
                               inputs["W2"], inputs["b2"])
    wk2eff, bk2e, s2k = _fold_mlp(inputs["Wk1"], inputs["gk"], inputs["bk"],
                                  inputs["Wk2"], inputs["bk2"])
    w1_h = np.asarray(inputs["W1"], np.float32)
    wk1_h = np.asarray(inputs["Wk1"], np.float32)
    # fast-path constants (om == 0): h = wk * leaky(Wk1[1]); see docstring
    lb = np.where(wk1_h[1] > 0, wk1_h[1], cfg.neg_slope * wk1_h[1]).astype(
        np.float64)
    m0 = lb.mean()
    v0 = (lb ** 2).mean() - m0 ** 2
    d0 = float(np.dot(lb, wk2eff.astype(np.float64)))
    g0 = d0 - m0 * float(s2k)

    cst = np.zeros(16, np.float32)
    cst[0], cst[1], cst[2], cst[3] = -s2, b2e, -s2k, bk2e
    cst[4] = np.float32(cfg.ln_eps)
    cst[5] = np.float32(v0)
    cst[6] = np.float32(g0)
    # wpack layout: [0:od] W1[0:128] | [od:2od] W1[128:256] | [2od:3od] w2eff
    # | [3od:4od] wk2eff | [4od:4od+16] cst | [4od+16:5od+16] Wk1 (rows 0-1)
    wcols = 5 * od + 16
    wpack = np.zeros((P, wcols), np.float32)
    wpack[:, 0:od] = w1_h[0:P]
    wpack[:, od:2 * od] = w1_h[P:2 * P]
    wpack[:, 2 * od:3 * od] = np.broadcast_to(w2eff, (P, od))
    wpack[:, 3 * od:4 * od] = np.broadcast_to(wk2eff, (P, od))
    wpack[:, 4 * od:4 * od + 16] = np.broadcast_to(cst, (P, 16))
    wpack[0:2, 4 * od + 16:5 * od + 16] = wk1_h

    # ---- phase C sharding (equal row count, target-aligned)
    tot1 = int(start1[-1])
    tgb = [0]
    for c in range(1, NCORES):
        tgb.append(int(np.searchsorted(start1, tot1 * c // NCORES)))
    tgb.append(cfg.len_edges)
    c_streams = []
    end_global = np.full(cfg.len_edges, -1, np.int64)
    blk = P * cfg.t2
    for c in range(NCORES):
        lo1, hi1 = tgb[c], tgb[c + 1]
        g1lo, g1hi = int(start1[lo1]), int(start1[hi1])
        r_sel = order1[g1lo:g1hi]
        tgtl = (tgts[g1lo:g1hi] - lo1)
        tcounts = cnt1[lo1:hi1]
        tstarts = (start1[lo1:hi1] - g1lo)
        st2 = _layout_stream(
            {"csrc": (src[r_sel].astype(np.int32), sent)},
            tgtl, tcounts, tstarts, cfg.t2)
        c_streams.append(st2)
        nz = tcounts > 0
        end_global[lo1:hi1][nz] = (st2["__end"].astype(np.int64)[nz]
                                   + c * blk)
    sentinel_slot = blk - 1
    # ---- phase D token split (nonempty -> full MLP, empty -> fast path)
    ne = cnt1 > 0
    T_ne = np.nonzero(ne)[0]
    T_e = np.nonzero(~ne)[0]
    per_f = -(-len(T_ne) // NCORES) if len(T_ne) else 0
    per_e = -(-len(T_e) // NCORES) if len(T_e) else 0
    assert per_f <= cfg.tokd_full, (per_f, cfg.tokd_full)
    assert per_e <= cfg.tokd, (per_e, cfg.tokd)
    asm = {"T_ne": T_ne, "T_e": T_e, "per_f": per_f, "per_e": per_e}

    in_maps = []
    for c in range(NCORES):
        m = {}
        m["xT"] = np.ascontiguousarray(
            xT[:, c * cfg.toka:(c + 1) * cfg.toka])
        m["wpack"] = wpack
        # --- B
        lo, hi = c * seg_pc, (c + 1) * seg_pc
        glo, ghi = int(start0[lo]), int(start0[hi])
        e_sel = order0[glo:ghi]
        segl = (seg0s[glo:ghi] - lo)
        scounts = cnt0[lo:hi]
        sstarts = (start0[lo:hi] - glo)
        pos1 = tab_pos(seg1[e_sel]).astype(np.int32)
        st = _layout_stream(
            {"bp": (pce[e_sel], np.float32(-300.0)),
             "bpos": (pos1, sent)},
            segl, scounts, sstarts, cfg.t1)
        m["bp"] = st["bp"]
        m["bpos"] = st["bpos"]
        m["bmsk"] = st["__mask"]
        m["bidx"] = st["__end"].reshape(P, seg_pc // P)
        # --- C
        st2 = c_streams[c]
        m["csrc"] = st2["csrc"]
        m["cmsk"] = st2["__mask"]
        # --- D full path
        tne = T_ne[c * per_f:(c + 1) * per_f]
        nf = len(tne)
        cidxF = np.full(cfg.tokd_full, sentinel_slot, np.int64)
        cidxF[:nf] = end_global[tne]
        m["cidxf"] = cidxF.astype(np.int32).reshape(P, cfg.tokd_full // P)
        wkF = np.zeros(cfg.tokd_full, np.float32)
        wkF[:nf] = wk[tne]
        m["wkf"] = wkF
        mF = np.zeros(cfg.tokd_full, np.float32)
        mF[:nf] = 1.0
        # yF lives in matmul slot order: token ch*d_chunk + sub*128 + p sits
        # at (p, ch*dsub + sub); lay the mask out to match
        nch = cfg.tokd_full // cfg.d_chunk
        dsub = cfg.d_chunk // P
        m["mf"] = np.ascontiguousarray(
            mF.reshape(nch, dsub, P).transpose(2, 0, 1).reshape(
                P, cfg.tokd_full // P))
        # --- D fast path
        te = T_e[c * per_e:(c + 1) * per_e]
        nE = len(te)
        wkE = np.zeros(cfg.tokd, np.float32)
        wkE[:nE] = wk[te]
        m["wke"] = wkE.reshape(P, cfg.tokd // P)
        mE = np.zeros(cfg.tokd, np.float32)
        mE[:nE] = 1.0
        m["me"] = mE.reshape(P, cfg.tokd // P)
        in_maps.append(m)
    return in_maps, asm


def assemble_output(results, asm, cfg):
    T_ne, T_e = asm["T_ne"], asm["T_e"]
    per_f, per_e = asm["per_f"], asm["per_e"]
    nch = cfg.tokd_full // cfg.d_chunk
    subs = cfg.d_chunk // P
    out = np.empty(cfg.len_edges, np.float32)
    for c in range(NCORES):
        yf = np.asarray(results[c]["y"]).reshape(P, nch, subs)
        yf = yf.transpose(1, 2, 0).reshape(-1)
        tne = T_ne[c * per_f:(c + 1) * per_f]
        out[tne] = yf[:len(tne)]
        ye = np.asarray(results[c]["y2"]).reshape(-1)
        te = T_e[c * per_e:(c + 1) * per_e]
        out[te] = ye[:len(te)]
    return out


# ---------------------------------------------------------------------------
# device program
# ---------------------------------------------------------------------------

def build_nc(cfg):
    import concourse.bass as bass
    import concourse.bacc as bacc
    import concourse.mybir as mybir
    import concourse.tile as tile
    from contextlib import ExitStack

    dt = mybir.dt
    f32, i32 = dt.float32, dt.int32
    hdt = dt.bfloat16 if cfg.mlp_bf16 else dt.float32
    Alu = mybir.AluOpType
    Act = mybir.ActivationFunctionType

    n_cyc, od = cfg.n_cyc, cfg.out_dim
    seg_pc = cfg.seg_pc
    toka, tokd, tokdf = cfg.toka, cfg.tokd, cfg.tokd_full
    t1, t2 = cfg.t1, cfg.t2
    kch = od // P
    spr = toka // P
    bcols = seg_pc // P
    fcols = tokdf // P
    ecols = tokd // P
    groups = [list(range(NCORES))]

    nc = bacc.Bacc(trn_type="TRN2", num_devices=NCORES)

    def din(name, shape, dtype=f32):
        return nc.declare_dram_parameter(name, list(shape), dtype, False).ap()

    wcols = 5 * od + 16
    xT = din("xT", [od, toka])
    wpack = din("wpack", [P, wcols])
    bp = din("bp", [P, t1])
    bpos = din("bpos", [P, t1], i32)
    bmsk = din("bmsk", [P, t1])
    bidx = din("bidx", [P, bcols], i32)
    csrc = din("csrc", [P, t2], i32)
    cmsk = din("cmsk", [P, t2])
    cidxf = din("cidxf", [P, fcols], i32)
    wkf = din("wkf", [tokdf])
    mf = din("mf", [P, fcols])
    wke = din("wke", [P, ecols])
    me = din("me", [P, ecols])
    y_out = nc.declare_dram_parameter("y", [tokdf], f32, True).ap()
    y2_out = nc.declare_dram_parameter("y2", [tokd], f32, True).ap()

    out_part = nc.dram_tensor("out_part", [toka], f32).ap()
    out_tab = nc.dram_tensor("out_tab", [n_cyc + 16], f32,
                             addr_space="Shared").ap()
    bredW = nc.dram_tensor("bredW", [P * t1], f32).ap()
    bredU = nc.dram_tensor("bredU", [P * t1], f32).ap()
    out2_part = nc.dram_tensor("out2_part", [seg_pc], f32).ap()
    out2_tab = nc.dram_tensor("out2_tab", [n_cyc + 16], f32,
                              addr_space="Shared").ap()
    credM = nc.dram_tensor("credM", [P * t2], f32).ap()
    credM_all = nc.dram_tensor("credM_all", [NCORES * P * t2], f32,
                               addr_space="Shared").ap()
    featT = nc.dram_tensor("featT", [2, tokdf], f32).ap()
    nsq_part = nc.dram_tensor("nsq_part", [16], f32).ap()
    nsq_tab = nc.dram_tensor("nsq_tab", [16], f32, addr_space="Shared").ap()

    def r2(ap_, p=P):
        return ap_.rearrange("(p c) -> p c", p=p)

    def col(ap_):
        return ap_.rearrange("(a b) -> a b", b=1)

    def _finish(ctx):
        ctx.close()
        return nc

    with ExitStack() as ctx:
        tc = ctx.enter_context(tile.TileContext(nc))
        cpool = ctx.enter_context(tc.tile_pool(name="cpool", bufs=1))
        sb = ctx.enter_context(tc.tile_pool(name="sb", bufs=2))
        sb3 = ctx.enter_context(tc.tile_pool(name="sb3", bufs=3))
        ps = ctx.enter_context(tc.tile_pool(name="ps", bufs=4, space="PSUM"))
        ps1 = ctx.enter_context(tc.tile_pool(name="ps1", bufs=1,
                                             space="PSUM"))

        def stt(out, in0, scalar, in1, op0, op1, accum=None):
            nc.vector.scalar_tensor_tensor(out=out, in0=in0, scalar=scalar,
                                           in1=in1, op0=op0, op1=op1,
                                           accum_out=accum)

        # --- fence: the ISA caps sync-wait commands per instruction.  DMA
        # lanes touched since the last fence are funneled into per-engine
        # nop chains (each nop carries <=2 lane waits) so every engine's
        # observed vector clock covers them and later instructions need no
        # direct lane waits.
        from concourse.tile import add_dep_helper as _adh
        loose = []

        def DMA(*a, **kw):
            inst = nc.sync.dma_start(*a, **kw)
            loose.append(inst)
            return inst

        def IDMA(*a, **kw):
            inst = nc.gpsimd.indirect_dma_start(*a, **kw)
            loose.append(inst)
            return inst

        def CC(*a, **kw):
            inst = nc.gpsimd.collective_compute(*a, **kw)
            loose.append(inst)
            return inst

        def fence():
            items = list(loose)
            loose.clear()
            if not items:
                return
            for eng in (nc.vector, nc.scalar, nc.tensor, nc.gpsimd,
                        nc.sync):
                for j in range(0, len(items), 2):
                    nop = eng.nop()
                    for d in items[j:j + 2]:
                        _adh(nop.ins, d.ins, sync=True, reason="fence")
            tc.no_sync_barrier()

        # ---- constants: one DMA
        wp = cpool.tile([P, wcols], f32, tag="wp")
        DMA(out=wp[:], in_=wpack[:, :])
        w1_sb = [wp[:, k * od:(k + 1) * od] for k in range(kch)]
        cst = wp[:, 4 * od:4 * od + 16]
        wk1_sb = wp[0:2, 4 * od + 16:5 * od + 16]
        w2b_sb = cpool.tile([P, od], hdt, tag="w2b")
        nc.vector.tensor_copy(out=w2b_sb[:], in_=wp[:, 2 * od:3 * od])
        wk2b_sb = cpool.tile([P, od], hdt, tag="wk2b")
        nc.vector.tensor_copy(out=wk2b_sb[:], in_=wp[:, 3 * od:4 * od])
        sent0 = cpool.tile([1, 16], f32, tag="sent0")
        nc.gpsimd.memset(sent0[:], 0.0)
        sent1 = cpool.tile([1, 16], f32, tag="sent1")
        nc.gpsimd.memset(sent1[:], -1.0e30)

        # shared MLP->scalar block ------------------------------------
        def mlp_block(nsub, lhsT_for, neg_s2_col, b2e_col, w2eff_tile,
                      out_cols):
            S = sb.tile([P, nsub], f32, tag="mlpS")
            Q = sb.tile([P, nsub], f32, tag="mlpQ")
            D_ = sb.tile([P, nsub], f32, tag="mlpD")
            for s in range(nsub):
                pst = ps.tile([P, od], f32, tag="mlp_ps")
                pieces = lhsT_for(s)
                for i, (lt, wt) in enumerate(pieces):
                    nc.tensor.matmul(out=pst[:], lhsT=lt, rhs=wt,
                                     start=(i == 0),
                                     stop=(i == len(pieces) - 1))
                h = sb3.tile([P, od], hdt, tag="mlp_h")
                nc.scalar.activation(out=h[:], in_=pst[:], func=Act.Lrelu,
                                     alpha=cfg.neg_slope,
                                     accum_out=S[:, s:s + 1])
                hsq = sb3.tile([P, od], hdt, tag="mlp_hsq")
                stt(hsq[:], h[:], 1.0, h[:], Alu.mult, Alu.mult,
                    accum=Q[:, s:s + 1])
                dsc = sb3.tile([P, od], hdt, tag="mlp_dsc")
                stt(dsc[:], h[:], 1.0, w2eff_tile[:], Alu.mult, Alu.mult,
                    accum=D_[:, s:s + 1])
            mu = sb.tile([P, nsub], f32, tag="mlp_mu")
            nc.vector.tensor_scalar_mul(out=mu[:], in0=S[:],
                                        scalar1=1.0 / od)
            var = sb.tile([P, nsub], f32, tag="mlp_var")
            stt(var[:], mu[:], 1.0, mu[:], Alu.mult, Alu.mult)
            qn = sb.tile([P, nsub], f32, tag="mlp_qn")
            nc.vector.tensor_scalar_mul(out=qn[:], in0=Q[:],
                                        scalar1=1.0 / od)
            nc.vector.tensor_tensor(out=var[:], in0=qn[:], in1=var[:],
                                    op=Alu.subtract)
            sd = sb.tile([P, nsub], f32, tag="mlp_sd")
            nc.scalar.activation(out=sd[:], in_=var[:], func=Act.Sqrt,
                                 bias=cst[:, 4:5])
            rs = sb.tile([P, nsub], f32, tag="mlp_rs")
            nc.vector.reciprocal(out=rs[:], in_=sd[:])
            tmp = sb.tile([P, nsub], f32, tag="mlp_tmp")
            stt(tmp[:], mu[:], neg_s2_col, D_[:], Alu.mult, Alu.add)
            nc.vector.tensor_tensor(out=tmp[:], in0=tmp[:], in1=rs[:],
                                    op=Alu.mult)
            stt(out_cols, tmp[:], 1.0, b2e_col.to_broadcast([P, nsub]),
                Alu.mult, Alu.add)

        fence()

        # ============================================================
        # Phase A
        # ============================================================
        out_sb = cpool.tile([P, spr], f32, tag="out_sb")
        nmac = toka // cfg.a_mac
        msub = cfg.a_mac // P
        xTk = xT.rearrange("(k p) c -> p k c", k=kch)
        for mblk in range(nmac):
            xt = sb.tile([P, kch * cfg.a_mac], f32, tag="xt")
            DMA(
                out=xt[:],
                in_=xTk[:, :, mblk * cfg.a_mac:(mblk + 1) * cfg.a_mac])

            def lhsT_a(s, _xt=xt):
                return [(_xt[:, k * cfg.a_mac + s * P:
                             k * cfg.a_mac + (s + 1) * P], w1_sb[k])
                        for k in range(kch)]

            mlp_block(msub, lhsT_a, cst[:, 0:1], cst[:, 1:2], w2b_sb,
                      out_sb[:, mblk * msub:(mblk + 1) * msub])
        DMA(out=r2(out_part), in_=out_sb[:])

        CC(
            "AllGather", Alu.bypass, replica_groups=groups,
            ins=[out_part[:]], outs=[out_tab[0:n_cyc]])
        DMA(out=r2(out_tab[n_cyc:n_cyc + 16], p=1),
                          in_=sent0[:])
        fence()

        def stub_outputs():
            dumf = cpool.tile([P, fcols], f32, tag="dumf")
            nc.gpsimd.memset(dumf[:], 0.0)
            dume = cpool.tile([P, ecols], f32, tag="dume")
            nc.gpsimd.memset(dume[:], 0.0)
            DMA(out=r2(y_out), in_=dumf[:])
            DMA(out=r2(y2_out), in_=dume[:])

        if "b" not in cfg.phases:
            stub_outputs()
            return _finish(ctx)

        # ============================================================
        # Phase B
        # ============================================================
        gc = t1 // cfg.gather_chunks
        with tc.tile_pool(name="bpool", bufs=3) as bpl, \
                tc.tile_pool(name="bstage", bufs=1) as bst:
            wstage = bst.tile([P, t1], f32, tag="wstage")
            ustage = bst.tile([P, t1], f32, tag="ustage")
            for i in range(cfg.gather_chunks):
                sl = slice(i * gc, (i + 1) * gc)
                bpc = bpl.tile([P, gc], f32, tag="bpc")
                DMA(out=bpc[:], in_=bp[:, sl])
                bmc = bpl.tile([P, gc], f32, tag="bmc")
                DMA(out=bmc[:], in_=bmsk[:, sl])
                bposc = bpl.tile([P, gc], i32, tag="bposc")
                DMA(out=bposc[:], in_=bpos[:, sl])
                gct = bpl.tile([P, gc], f32, tag="gct")
                IDMA(
                    out=gct[:], out_offset=None, in_=col(out_tab[:]),
                    in_offset=bass.IndirectOffsetOnAxis(ap=bposc[:], axis=0))
                stt(bpc[:], bpc[:], cfg.neg_slope, bpc[:], Alu.mult, Alu.max)
                nc.scalar.activation(out=bpc[:], in_=bpc[:], func=Act.Exp)
                nc.vector.tensor_tensor(out=gct[:], in0=bpc[:], in1=gct[:],
                                        op=Alu.mult)
                nc.vector.tensor_tensor_scan(
                    out=wstage[:, sl], data0=bmc[:], data1=bpc[:],
                    initial=(0.0 if i == 0 else
                             wstage[:, i * gc - 1:i * gc]),
                    op0=Alu.mult, op1=Alu.add)
                nc.vector.tensor_tensor_scan(
                    out=ustage[:, sl], data0=bmc[:], data1=gct[:],
                    initial=(0.0 if i == 0 else
                             ustage[:, i * gc - 1:i * gc]),
                    op0=Alu.mult, op1=Alu.add)
            DMA(out=r2(bredW), in_=wstage[:])
            DMA(out=r2(bredU), in_=ustage[:])
            fence()

        bidx_sb = cpool.tile([P, bcols], i32, tag="bidx_sb")
        DMA(out=bidx_sb[:], in_=bidx[:, :])
        dW = cpool.tile([P, bcols], f32, tag="dW")
        IDMA(
            out=dW[:], out_offset=None, in_=col(bredW[:]),
            in_offset=bass.IndirectOffsetOnAxis(ap=bidx_sb[:], axis=0))
        dU = cpool.tile([P, bcols], f32, tag="dU")
        IDMA(
            out=dU[:], out_offset=None, in_=col(bredU[:]),
            in_offset=bass.IndirectOffsetOnAxis(ap=bidx_sb[:], axis=0))
        o2 = cpool.tile([P, bcols], f32, tag="o2")
        nc.vector.tensor_scalar_add(out=o2[:], in0=dW[:], scalar1=1e-30)
        nc.vector.reciprocal(out=o2[:], in_=o2[:])
        nc.vector.tensor_tensor(out=o2[:], in0=o2[:], in1=dU[:], op=Alu.mult)
        nc.vector.tensor_scalar_add(out=o2[:], in0=o2[:],
                                    scalar1=cfg.cshift)
        DMA(out=r2(out2_part), in_=o2[:])
        fence()
        CC(
            "AllGather", Alu.bypass, replica_groups=groups,
            ins=[out2_part[:]], outs=[out2_tab[0:n_cyc]])
        DMA(out=r2(out2_tab[n_cyc:n_cyc + 16], p=1),
                          in_=sent1[:])
        fence()

        if "c" not in cfg.phases:
            stub_outputs()
            return _finish(ctx)

        # ============================================================
        # Phase C
        # ============================================================
        gc2 = t2 // cfg.gather_chunks
        with tc.tile_pool(name="cpool2", bufs=3) as cpl, \
                tc.tile_pool(name="cstage", bufs=1) as cstg:
            mstage = cstg.tile([P, t2], f32, tag="mstage")
            for i in range(cfg.gather_chunks):
                sl = slice(i * gc2, (i + 1) * gc2)
                cmc = cpl.tile([P, gc2], f32, tag="cmc")
                DMA(out=cmc[:], in_=cmsk[:, sl])
                csc = cpl.tile([P, gc2], i32, tag="csc")
                DMA(out=csc[:], in_=csrc[:, sl])
                g2t = cpl.tile([P, gc2], f32, tag="g2t")
                IDMA(
                    out=g2t[:], out_offset=None, in_=col(out2_tab[:]),
                    in_offset=bass.IndirectOffsetOnAxis(ap=csc[:], axis=0))
                nc.vector.tensor_tensor_scan(
                    out=mstage[:, sl], data0=cmc[:], data1=g2t[:],
                    initial=(0.0 if i == 0 else
                             mstage[:, i * gc2 - 1:i * gc2]),
                    op0=Alu.mult, op1=Alu.max)
            DMA(out=r2(credM), in_=mstage[:])
            fence()
        CC(
            "AllGather", Alu.bypass, replica_groups=groups,
            ins=[credM[:]], outs=[credM_all[:]])
        fence()

        cidx_sb = cpool.tile([P, fcols], i32, tag="cidx_sb")
        DMA(out=cidx_sb[:], in_=cidxf[:, :])
        omr = cpool.tile([P, fcols], f32, tag="omr")
        IDMA(
            out=omr[:], out_offset=None, in_=col(credM_all[:]),
            in_offset=bass.IndirectOffsetOnAxis(ap=cidx_sb[:], axis=0))
        omm = cpool.tile([P, fcols], f32, tag="omm")
        nc.vector.tensor_scalar(out=omm[:], in0=omr[:], scalar1=32.0,
                                scalar2=None, op0=Alu.is_gt)
        om = cpool.tile([P, fcols], f32, tag="om")
        stt(om[:], omr[:], -cfg.cshift, omm[:], Alu.add, Alu.mult)
        DMA(out=r2(featT[0, :]), in_=om[:])
        wkb = cpool.tile([P, fcols], f32, tag="wkb")
        DMA(out=wkb[:], in_=r2(wkf))
        DMA(out=r2(featT[1, :]), in_=wkb[:])
        fence()

        if "d" not in cfg.phases:
            stub_outputs()
            return _finish(ctx)

        # ============================================================
        # Phase D: full MLP on nonempty targets
        # ============================================================
        yF = cpool.tile([P, fcols], f32, tag="yF")
        nchunk = tokdf // cfg.d_chunk
        dsub = cfg.d_chunk // P
        with tc.tile_pool(name="dpool", bufs=2) as dpl:
            for ch in range(nchunk):
                ft = dpl.tile([2, cfg.d_chunk], f32, tag="ft")
                DMA(
                    out=ft[:],
                    in_=featT[:, ch * cfg.d_chunk:(ch + 1) * cfg.d_chunk])

                def lhsT_d(s, _ft=ft):
                    return [(_ft[:, s * P:(s + 1) * P], wk1_sb)]

                mlp_block(dsub, lhsT_d, cst[:, 2:3], cst[:, 3:4], wk2b_sb,
                          yF[:, ch * dsub:(ch + 1) * dsub])

        fence()

        # ---- fast path: empty targets (om == 0)
        wke_sb = cpool.tile([P, ecols], f32, tag="wke_sb")
        DMA(out=wke_sb[:], in_=wke[:, :])
        wk2t = cpool.tile([P, ecols], f32, tag="wk2t")
        stt(wk2t[:], wke_sb[:], 1.0, wke_sb[:], Alu.mult, Alu.mult)
        nc.vector.tensor_scalar(out=wk2t[:], in0=wk2t[:],
                                scalar1=cst[:, 5:6], scalar2=cst[:, 4:5],
                                op0=Alu.mult, op1=Alu.add)
        nc.scalar.activation(out=wk2t[:], in_=wk2t[:], func=Act.Sqrt)
        nc.vector.reciprocal(out=wk2t[:], in_=wk2t[:])
        yE = cpool.tile([P, ecols], f32, tag="yE")
        nc.vector.tensor_scalar(out=yE[:], in0=wke_sb[:],
                                scalar1=cst[:, 6:7], scalar2=None,
                                op0=Alu.mult)
        nc.vector.tensor_tensor(out=yE[:], in0=yE[:], in1=wk2t[:],
                                op=Alu.mult)
        nc.vector.tensor_scalar(out=yE[:], in0=yE[:], scalar1=cst[:, 3:4],
                                scalar2=None, op0=Alu.add)

        # ---- global L2 norm
        mf_sb = cpool.tile([P, fcols], f32, tag="mf_sb")
        DMA(out=mf_sb[:], in_=mf[:, :])
        me_sb = cpool.tile([P, ecols], f32, tag="me_sb")
        DMA(out=me_sb[:], in_=me[:, :])
        ssq = cpool.tile([P, 2], f32, tag="ssq")
        scrF = cpool.tile([P, fcols], f32, tag="scrF")
        nc.vector.tensor_tensor(out=scrF[:], in0=yF[:], in1=mf_sb[:],
                                op=Alu.mult)
        scrF2 = cpool.tile([P, fcols], f32, tag="scrF2")
        stt(scrF2[:], scrF[:], 1.0, yF[:], Alu.mult, Alu.mult,
            accum=ssq[:, 0:1])
        scrE = cpool.tile([P, ecols], f32, tag="scrE")
        nc.vector.tensor_tensor(out=scrE[:], in0=yE[:], in1=me_sb[:],
                                op=Alu.mult)
        scrE2 = cpool.tile([P, ecols], f32, tag="scrE2")
        stt(scrE2[:], scrE[:], 1.0, yE[:], Alu.mult, Alu.mult,
            accum=ssq[:, 1:2])
        ssqt = cpool.tile([P, 1], f32, tag="ssqt")
        nc.vector.tensor_tensor(out=ssqt[:], in0=ssq[:, 0:1],
                                in1=ssq[:, 1:2], op=Alu.add)
        ones = cpool.tile([P, 1], f32, tag="ones")
        nc.gpsimd.memset(ones[:], 1.0)
        sred = ps1.tile([1, 1], f32, tag="sred")
        nc.tensor.matmul(out=sred[:], lhsT=ones[:], rhs=ssqt[:],
                         start=True, stop=True)
        nsq_sb = cpool.tile([1, 16], f32, tag="nsq_sb")
        nc.gpsimd.memset(nsq_sb[:], 0.0)
        nc.vector.tensor_copy(out=nsq_sb[:, 0:1], in_=sred[:])
        DMA(out=r2(nsq_part, p=1), in_=nsq_sb[:])
        fence()
        CC(
            "AllReduce", Alu.add, replica_groups=groups,
            ins=[nsq_part[:]], outs=[nsq_tab[:]])
        fence()
        nrm = cpool.tile([1, 1], f32, tag="nrm")
        DMA(out=nrm[:], in_=r2(nsq_tab[0:1], p=1))
        nc.scalar.activation(out=nrm[:], in_=nrm[:], func=Act.Sqrt)
        nc.vector.tensor_scalar_max(out=nrm[:], in0=nrm[:], scalar1=1e-12)
        nc.vector.reciprocal(out=nrm[:], in_=nrm[:])
        ones_row = cpool.tile([1, P], f32, tag="ones_row")
        nc.gpsimd.memset(ones_row[:], 1.0)
        rn_ps = ps1.tile([P, 1], f32, tag="rn_ps")
        nc.tensor.matmul(out=rn_ps[:], lhsT=ones_row[:], rhs=nrm[:],
                         start=True, stop=True)
        rn_sb = cpool.tile([P, 1], f32, tag="rn_sb")
        nc.vector.tensor_copy(out=rn_sb[:], in_=rn_ps[:])
        # sigmoid(x) = 1/(1+exp(-x)) via Exp + HW reciprocal: the ACT
        # sigmoid table's absolute error lands directly on the output.
        def scale_sigmoid(t):
            nc.scalar.activation(out=t, in_=t, func=Act.Exp,
                                 scale=nrn_sb[:, 0:1])
            nc.vector.tensor_scalar_add(out=t, in0=t, scalar1=1.0)
            nc.vector.reciprocal(out=t, in_=t)

        nrn_sb = cpool.tile([P, 1], f32, tag="nrn_sb")
        nc.vector.tensor_scalar_mul(out=nrn_sb[:], in0=rn_sb[:],
                                    scalar1=-1.0)
        scale_sigmoid(yF[:])
        DMA(out=r2(y_out), in_=yF[:])
        scale_sigmoid(yE[:])
        DMA(out=r2(y2_out), in_=yE[:])

    return nc


# ---------------------------------------------------------------------------
# entry point
# ---------------------------------------------------------------------------

_NC_CACHE = {}


def _get_nc(cfg):
    key = (cfg.n_cyc, cfg.e_cc, cfg.len_edges, cfg.t1, cfg.t2,
           cfg.tokd_full)
    if key not in _NC_CACHE:
        nc = build_nc(cfg)
        if not nc.is_finalized():
            nc.finalize()
        _NC_CACHE[key] = nc
    return _NC_CACHE[key]


def run(inputs, cfg=None, trace=False):
    from concourse.bass_utils import run_bass_kernel_spmd
    cfg = cfg or Cfg()
    in_maps, asm = host_prepare(inputs, cfg)
    nc = _get_nc(cfg)
    res = run_bass_kernel_spmd(nc, in_maps, core_ids=list(range(NCORES)),
                               trace=trace)
    return assemble_output(res.results, asm, cfg), res


def kernel(**inputs):
    out, _ = run(inputs)
    return out



# revision 13
# speedup vs baseline: 1.3448x; 1.3448x over previous
"""Trainium2 (Bass) SPMD kernel for the CBGNN message-passing problem. v4.

v4 changes over v3 (see kernel_v3_backup.py):
  - phase-D full-MLP tokens are owned by the core that owns the target's
    Edge2cycle rows in phase C, so the per-target maxima stay local: the
    17MB credM AllGather and the cross-core omr gather are gone;
  - phase-A matmuls run in bf16 (xT is shipped as bf16, W1 cast on
    device): fp32 matmul costs 4 PE cycles/row, bf16 costs 1;
  - out/out2 tables AllGather in bf16 (half the collective bytes); the
    +64 positivity shift for the phase-C max scan is dropped (a -1e29
    is_gt threshold separates real maxima from the -1e30 sentinel);
  - the h^2 accumulation in the MLP runs on GpSimd, taking it off the
    DVE critical path (DVE keeps only the h*w2eff accumulation);
  - whether_k staging (featT row 1) and the phase-D masks load before
    phase A so only featT row 0 is on the post-C critical path;
  - cfg.reps re-emits the whole body N times in one NEFF (timing
    amplification for the dev loop; the shipped config uses reps=1).
"""

import sys

import numpy as np

for _p in ("/opt/trn_rl_repo",):
    if _p not in sys.path:
        sys.path.insert(0, _p)

NCORES = 8
P = 128


class Cfg:
    n_cyc = 262144
    out_dim = 256
    e_cc = 4194304
    m_e2c = 4194304
    len_edges = 1048576
    t1 = 4224            # phase-B stream columns per partition row
    t2 = 4224            # phase-C stream columns per partition row
    a_mac = 2048         # phase-A macro tile (tokens)
    tokd_full = 36864    # phase-D full-MLP capacity per core
    d_chunk = 4608       # phase-D featT chunk (tokens)
    gather_chunks = 8    # indirect-gather splits
    neg_slope = 0.2
    ln_eps = 1e-5
    mlp_bf16 = True
    reps = 1
    phases = "abcd"

    @property
    def seg_pc(self):
        return self.n_cyc // NCORES

    @property
    def toka(self):
        return self.n_cyc // NCORES

    @property
    def tgt_pc(self):
        return self.len_edges // NCORES

    @property
    def tokd(self):
        return self.len_edges // NCORES


class SmallCfg(Cfg):
    n_cyc = 16384
    e_cc = 65536
    m_e2c = 65536
    len_edges = 16384
    t1 = 96
    t2 = 96
    a_mac = 2048
    tokd_full = 2304
    d_chunk = 2304
    gather_chunks = 2


# ---------------------------------------------------------------------------
# host-side sharding / layout (index work + parameter folding only)
# ---------------------------------------------------------------------------

def _pack_rows(counts, ncols):
    csum = np.cumsum(counts)
    nseg = len(counts)
    seg_row = np.empty(nseg, np.int64)
    seg_col0 = np.empty(nseg, np.int64)
    start = 0
    base = 0
    for r in range(P):
        j = int(np.searchsorted(csum, base + ncols, side="right"))
        if j < nseg and counts[j] > ncols:
            raise ValueError("segment larger than a row")
        prev = base
        seg_row[start:j] = r
        seg_col0[start:j] = (csum[start:j] - counts[start:j]) - prev
        if j > 0:
            base = int(csum[j - 1])
        start = j
        if start == nseg:
            break
    if start != nseg:
        raise ValueError("edges did not fit into P rows")
    return seg_row, seg_col0


def _layout_stream(named_vals, seg_local, seg_counts, seg_starts_local, ncols):
    seg_row, seg_col0 = _pack_rows(seg_counts, ncols)
    rank = np.arange(len(seg_local)) - seg_starts_local[seg_local]
    slot = seg_row[seg_local] * ncols + seg_col0[seg_local] + rank
    out = {}
    for name, (vals, fill) in named_vals.items():
        arr = np.full(P * ncols, fill, dtype=np.asarray(vals).dtype)
        arr[slot] = vals
        out[name] = arr.reshape(P, ncols)
    lab = np.full(P * ncols, -1, np.int64)
    lab[slot] = seg_local
    lab2 = lab.reshape(P, ncols)
    msk = np.zeros((P, ncols), np.float32)
    msk[:, 1:] = ((lab2[:, 1:] == lab2[:, :-1]) & (lab2[:, 1:] >= 0)).astype(
        np.float32)
    out["__mask"] = msk
    end_slot = seg_row * ncols + seg_col0 + seg_counts - 1
    filler_slot = P * ncols - 1
    assert lab.reshape(-1)[filler_slot] == -1, "last slot is not filler"
    out["__end"] = np.where(seg_counts > 0, end_slot,
                            filler_slot).astype(np.int32)
    return out


def _fold_mlp(W1, g, b, W2, b2):
    W2 = np.asarray(W2, np.float64).reshape(-1)
    g = np.asarray(g, np.float64)
    b = np.asarray(b, np.float64)
    w2eff = g * W2
    b2eff = float(np.asarray(b2).reshape(-1)[0]) + float(np.dot(b, W2))
    return (w2eff.astype(np.float32), np.float32(b2eff),
            np.float32(w2eff.sum()))


def host_prepare(inputs, cfg):
    import ml_dtypes
    n_cyc, seg_pc = cfg.n_cyc, cfg.seg_pc
    od = cfg.out_dim

    x = np.asarray(inputs["x"], np.float32)
    e2c = np.asarray(inputs["Edge2cycle"])
    eidx = np.asarray(inputs["edge_index"])
    pce = np.asarray(inputs["permuteCE"], np.float32)
    wk = np.asarray(inputs["whether_k"], np.float32)
    assert float(wk.min()) >= 0.0, "fast path requires whether_k >= 0"

    seg0 = np.asarray(eidx[0], np.int64)
    seg1 = np.asarray(eidx[1], np.int64)

    xT = np.ascontiguousarray(x.T).astype(ml_dtypes.bfloat16)
    spr = cfg.toka // P

    def tab_pos(g):
        core = g // seg_pc
        loc = g - core * seg_pc
        return core * seg_pc + (loc % P) * spr + loc // P

    order0 = np.argsort(seg0, kind="stable")
    seg0s = seg0[order0]
    cnt0 = np.bincount(seg0, minlength=n_cyc).astype(np.int64)
    start0 = np.zeros(n_cyc + 1, np.int64)
    np.cumsum(cnt0, out=start0[1:])

    tgt = np.asarray(e2c[:, 0], np.int64)
    src = np.asarray(e2c[:, 1], np.int64)
    order1 = np.argsort(tgt, kind="stable")
    cnt1 = np.bincount(tgt, minlength=cfg.len_edges).astype(np.int64)
    start1 = np.zeros(cfg.len_edges + 1, np.int64)
    np.cumsum(cnt1, out=start1[1:])
    tgts = tgt[order1]

    sent = np.int32(n_cyc)

    # ---- parameter folding
    w2eff, b2e, s2 = _fold_mlp(inputs["W1"], inputs["g1"], inputs["b1"],
                               inputs["W2"], inputs["b2"])
    wk2eff, bk2e, s2k = _fold_mlp(inputs["Wk1"], inputs["gk"], inputs["bk"],
                                  inputs["Wk2"], inputs["bk2"])
    w1_h = np.asarray(inputs["W1"], np.float32)
    wk1_h = np.asarray(inputs["Wk1"], np.float32)
    # fast-path constants (om == 0): h = wk * leaky(Wk1[1]); see docstring
    lb = np.where(wk1_h[1] > 0, wk1_h[1], cfg.neg_slope * wk1_h[1]).astype(
        np.float64)
    m0 = lb.mean()
    v0 = (lb ** 2).mean() - m0 ** 2
    d0 = float(np.dot(lb, wk2eff.astype(np.float64)))
    g0 = d0 - m0 * float(s2k)

    cst = np.zeros(16, np.float32)
    cst[0], cst[1], cst[2], cst[3] = -s2, b2e, -s2k, bk2e
    cst[4] = np.float32(cfg.ln_eps)
    cst[5] = np.float32(v0)
    cst[6] = np.float32(g0)
    # wpack layout: [0:od] W1[0:128] | [od:2od] W1[128:256] | [2od:3od] w2eff
    # | [3od:4od] wk2eff | [4od:4od+16] cst | [4od+16:5od+16] Wk1 (rows 0-1)
    wcols = 5 * od + 16
    wpack = np.zeros((P, wcols), np.float32)
    wpack[:, 0:od] = w1_h[0:P]
    wpack[:, od:2 * od] = w1_h[P:2 * P]
    wpack[:, 2 * od:3 * od] = np.broadcast_to(w2eff, (P, od))
    wpack[:, 3 * od:4 * od] = np.broadcast_to(wk2eff, (P, od))
    wpack[:, 4 * od:4 * od + 16] = np.broadcast_to(cst, (P, 16))
    wpack[0:2, 4 * od + 16:5 * od + 16] = wk1_h

    # ---- phase C sharding (row-balanced, target-aligned core ranges)
    tot1 = int(start1[-1])
    tgb = [0]
    for c in range(1, NCORES):
        tgb.append(int(np.searchsorted(start1, tot1 * c // NCORES)))
    tgb.append(cfg.len_edges)
    sentinel_slot = P * cfg.t2 - 1

    # ---- phase D empty-target split (even across cores)
    ne = cnt1 > 0
    T_e = np.nonzero(~ne)[0]
    per_e = -(-len(T_e) // NCORES) if len(T_e) else 0
    assert per_e <= cfg.tokd, (per_e, cfg.tokd)
    tne_list = []

    in_maps = []
    for c in range(NCORES):
        m = {}
        m["xT"] = np.ascontiguousarray(
            xT[:, c * cfg.toka:(c + 1) * cfg.toka])
        m["wpack"] = wpack
        # --- B
        lo, hi = c * seg_pc, (c + 1) * seg_pc
        glo, ghi = int(start0[lo]), int(start0[hi])
        e_sel = order0[glo:ghi]
        segl = (seg0s[glo:ghi] - lo)
        scounts = cnt0[lo:hi]
        sstarts = (start0[lo:hi] - glo)
        pos1 = tab_pos(seg1[e_sel]).astype(np.int32)
        st = _layout_stream(
            {"bp": (pce[e_sel], np.float32(-300.0)),
             "bpos": (pos1, sent)},
            segl, scounts, sstarts, cfg.t1)
        m["bp"] = st["bp"]
        m["bpos"] = st["bpos"]
        m["bmsk"] = st["__mask"]
        m["bidx"] = st["__end"].reshape(P, seg_pc // P)
        # --- C (targets [lo1, hi1) fully owned by this core)
        lo1, hi1 = tgb[c], tgb[c + 1]
        g1lo, g1hi = int(start1[lo1]), int(start1[hi1])
        r_sel = order1[g1lo:g1hi]
        tgtl = (tgts[g1lo:g1hi] - lo1)
        tcounts = cnt1[lo1:hi1]
        tstarts = (start1[lo1:hi1] - g1lo)
        st2 = _layout_stream(
            {"csrc": (src[r_sel].astype(np.int32), sent)},
            tgtl, tcounts, tstarts, cfg.t2)
        m["csrc"] = st2["csrc"]
        m["cmsk"] = st2["__mask"]
        # --- D full path: this core's row-bearing targets, local end slots
        nz_loc = np.nonzero(tcounts > 0)[0]
        tne = nz_loc + lo1
        nf = len(tne)
        assert nf <= cfg.tokd_full, (c, nf, cfg.tokd_full)
        tne_list.append(tne)
        cidxF = np.full(cfg.tokd_full, sentinel_slot, np.int64)
        cidxF[:nf] = st2["__end"].astype(np.int64)[nz_loc]
        m["cidxf"] = cidxF.astype(np.int32).reshape(P, cfg.tokd_full // P)
        wkF = np.zeros(cfg.tokd_full, np.float32)
        wkF[:nf] = wk[tne]
        m["wkf"] = wkF
        mF = np.zeros(cfg.tokd_full, np.float32)
        mF[:nf] = 1.0
        # yF lives in matmul slot order: token ch*d_chunk + sub*128 + p sits
        # at (p, ch*dsub + sub); lay the mask out to match
        nch = cfg.tokd_full // cfg.d_chunk
        dsub = cfg.d_chunk // P
        m["mf"] = np.ascontiguousarray(
            mF.reshape(nch, dsub, P).transpose(2, 0, 1).reshape(
                P, cfg.tokd_full // P))
        # --- D fast path
        te = T_e[c * per_e:(c + 1) * per_e]
        nE = len(te)
        wkE = np.zeros(cfg.tokd, np.float32)
        wkE[:nE] = wk[te]
        m["wke"] = wkE.reshape(P, cfg.tokd // P)
        mE = np.zeros(cfg.tokd, np.float32)
        mE[:nE] = 1.0
        m["me"] = mE.reshape(P, cfg.tokd // P)
        in_maps.append(m)
    asm = {"tne": tne_list, "T_e": T_e, "per_e": per_e}
    return in_maps, asm


def assemble_output(results, asm, cfg):
    T_e, per_e = asm["T_e"], asm["per_e"]
    nch = cfg.tokd_full // cfg.d_chunk
    subs = cfg.d_chunk // P
    out = np.empty(cfg.len_edges, np.float32)
    for c in range(NCORES):
        yf = np.asarray(results[c]["y"]).reshape(P, nch, subs)
        yf = yf.transpose(1, 2, 0).reshape(-1)
        tne = asm["tne"][c]
        out[tne] = yf[:len(tne)]
        ye = np.asarray(results[c]["y2"]).reshape(-1)
        te = T_e[c * per_e:(c + 1) * per_e]
        out[te] = ye[:len(te)]
    return out


# ---------------------------------------------------------------------------
# device program
# ---------------------------------------------------------------------------

def build_nc(cfg):
    import concourse.bass as bass
    import concourse.bacc as bacc
    import concourse.mybir as mybir
    import concourse.tile as tile
    from contextlib import ExitStack

    dt = mybir.dt
    f32, i32 = dt.float32, dt.int32
    bf16 = dt.bfloat16
    hdt = dt.bfloat16 if cfg.mlp_bf16 else dt.float32
    Alu = mybir.AluOpType
    Act = mybir.ActivationFunctionType

    n_cyc, od = cfg.n_cyc, cfg.out_dim
    seg_pc = cfg.seg_pc
    toka, tokd, tokdf = cfg.toka, cfg.tokd, cfg.tokd_full
    t1, t2 = cfg.t1, cfg.t2
    kch = od // P
    spr = toka // P
    bcols = seg_pc // P
    fcols = tokdf // P
    ecols = tokd // P
    groups = [list(range(NCORES))]

    nc = bacc.Bacc(trn_type="TRN2", num_devices=NCORES)

    def din(name, shape, dtype=f32):
        return nc.declare_dram_parameter(name, list(shape), dtype, False).ap()

    wcols = 5 * od + 16
    xT = din("xT", [od, toka], bf16)
    wpack = din("wpack", [P, wcols])
    bp = din("bp", [P, t1])
    bpos = din("bpos", [P, t1], i32)
    bmsk = din("bmsk", [P, t1])
    bidx = din("bidx", [P, bcols], i32)
    csrc = din("csrc", [P, t2], i32)
    cmsk = din("cmsk", [P, t2])
    cidxf = din("cidxf", [P, fcols], i32)
    wkf = din("wkf", [tokdf])
    mf = din("mf", [P, fcols])
    wke = din("wke", [P, ecols])
    me = din("me", [P, ecols])
    y_out = nc.declare_dram_parameter("y", [tokdf], f32, True).ap()
    y2_out = nc.declare_dram_parameter("y2", [tokd], f32, True).ap()

    out_part = nc.dram_tensor("out_part", [toka], bf16).ap()
    out_tab = nc.dram_tensor("out_tab", [n_cyc + 16], bf16,
                             addr_space="Shared").ap()
    bredW = nc.dram_tensor("bredW", [P * t1], f32).ap()
    bredU = nc.dram_tensor("bredU", [P * t1], f32).ap()
    out2_part = nc.dram_tensor("out2_part", [seg_pc], bf16).ap()
    out2_tab = nc.dram_tensor("out2_tab", [n_cyc + 16], bf16,
                              addr_space="Shared").ap()
    credM = nc.dram_tensor("credM", [P * t2], f32).ap()
    featT = nc.dram_tensor("featT", [2, tokdf], f32).ap()
    nsq_part = nc.dram_tensor("nsq_part", [16], f32).ap()
    nsq_tab = nc.dram_tensor("nsq_tab", [16], f32, addr_space="Shared").ap()

    def r2(ap_, p=P):
        return ap_.rearrange("(p c) -> p c", p=p)

    def col(ap_):
        return ap_.rearrange("(a b) -> a b", b=1)

    with ExitStack() as ctx:
        tc = ctx.enter_context(tile.TileContext(nc))
        cpool = ctx.enter_context(tc.tile_pool(name="cpool", bufs=1))
        sb = ctx.enter_context(tc.tile_pool(name="sb", bufs=2))
        sb3 = ctx.enter_context(tc.tile_pool(name="sb3", bufs=3))
        ps = ctx.enter_context(tc.tile_pool(name="ps", bufs=4, space="PSUM"))
        ps1 = ctx.enter_context(tc.tile_pool(name="ps1", bufs=1,
                                             space="PSUM"))

        def stt(out, in0, scalar, in1, op0, op1, accum=None):
            nc.vector.scalar_tensor_tensor(out=out, in0=in0, scalar=scalar,
                                           in1=in1, op0=op0, op1=op1,
                                           accum_out=accum)

        # --- fence: the ISA caps sync-wait commands per instruction.  DMA
        # lanes touched since the last fence are funneled into per-engine
        # nop chains (each nop carries <=2 lane waits) so every engine's
        # observed vector clock covers them and later instructions need no
        # direct lane waits.
        from concourse.tile import add_dep_helper as _adh
        loose = []

        def DMA(*a, **kw):
            inst = nc.sync.dma_start(*a, **kw)
            loose.append(inst)
            return inst

        def IDMA(*a, **kw):
            inst = nc.gpsimd.indirect_dma_start(*a, **kw)
            loose.append(inst)
            return inst

        def CC(*a, **kw):
            inst = nc.gpsimd.collective_compute(*a, **kw)
            loose.append(inst)
            return inst

        def fence():
            items = list(loose)
            loose.clear()
            if not items:
                return
            for eng in (nc.vector, nc.scalar, nc.tensor, nc.gpsimd,
                        nc.sync):
                for j in range(0, len(items), 2):
                    nop = eng.nop()
                    for d in items[j:j + 2]:
                        _adh(nop.ins, d.ins, sync=True, reason="fence")
            tc.no_sync_barrier()

        def emit_body():
            # ---- constants: one DMA
            wp = cpool.tile([P, wcols], f32, tag="wp")
            DMA(out=wp[:], in_=wpack[:, :])
            cst = wp[:, 4 * od:4 * od + 16]
            wk1_sb = wp[0:2, 4 * od + 16:5 * od + 16]
            w1b = cpool.tile([P, 2 * od], bf16, tag="w1b")
            nc.vector.tensor_copy(out=w1b[:], in_=wp[:, 0:2 * od])
            w1_sb = [w1b[:, k * od:(k + 1) * od] for k in range(kch)]
            w2b_sb = cpool.tile([P, od], hdt, tag="w2b")
            nc.vector.tensor_copy(out=w2b_sb[:], in_=wp[:, 2 * od:3 * od])
            wk2b_sb = cpool.tile([P, od], hdt, tag="wk2b")
            nc.vector.tensor_copy(out=wk2b_sb[:], in_=wp[:, 3 * od:4 * od])
            sent0 = cpool.tile([1, 16], bf16, tag="sent0")
            nc.gpsimd.memset(sent0[:], 0.0)
            sent1 = cpool.tile([1, 16], bf16, tag="sent1")
            nc.gpsimd.memset(sent1[:], -1.0e30)

            # ---- early staging: whether_k row of featT + phase-D masks
            wkb = cpool.tile([P, fcols], f32, tag="wkb")
            DMA(out=wkb[:], in_=r2(wkf))
            DMA(out=r2(featT[1, :]), in_=wkb[:])
            mf_sb = cpool.tile([P, fcols], f32, tag="mf_sb")
            DMA(out=mf_sb[:], in_=mf[:, :])
            me_sb = cpool.tile([P, ecols], f32, tag="me_sb")
            DMA(out=me_sb[:], in_=me[:, :])
            wke_sb = cpool.tile([P, ecols], f32, tag="wke_sb")
            DMA(out=wke_sb[:], in_=wke[:, :])
            cidx_sb = cpool.tile([P, fcols], i32, tag="cidx_sb")
            DMA(out=cidx_sb[:], in_=cidxf[:, :])
            bidx_sb = cpool.tile([P, bcols], i32, tag="bidx_sb")
            DMA(out=bidx_sb[:], in_=bidx[:, :])

            # shared MLP blocks -------------------------------------------
            # accumulate S/Q/D columns per 128-token subtile; Sqrt-based
            # finishing is batched per phase so the ACT table loads once.
            def mlp_accum(nsub, lhsT_for, S, Q, D_, col0, w2t):
                for s in range(nsub):
                    pst = ps.tile([P, od], f32, tag="mlp_ps")
                    pieces = lhsT_for(s)
                    for i, (lt, wt) in enumerate(pieces):
                        nc.tensor.matmul(out=pst[:], lhsT=lt, rhs=wt,
                                         start=(i == 0),
                                         stop=(i == len(pieces) - 1))
                    c = col0 + s
                    h = sb3.tile([P, od], hdt, tag="mlp_h")
                    nc.scalar.activation(out=h[:], in_=pst[:], func=Act.Lrelu,
                                         alpha=cfg.neg_slope,
                                         accum_out=S[:, c:c + 1])
                    hsq = sb3.tile([P, od], hdt, tag="mlp_hsq")
                    stt(hsq[:], h[:], 1.0, h[:], Alu.mult, Alu.mult,
                        accum=Q[:, c:c + 1])
                    dsc = sb3.tile([P, od], hdt, tag="mlp_dsc")
                    stt(dsc[:], h[:], 1.0, w2t[:], Alu.mult, Alu.mult,
                        accum=D_[:, c:c + 1])

            def mlp_finish(S, Q, D_, ncols, neg_s2_col, b2e_col, out_tile):
                mu = sb.tile([P, ncols], f32, tag="mlp_mu")
                nc.vector.tensor_scalar_mul(out=mu[:], in0=S[:],
                                            scalar1=1.0 / od)
                var = sb.tile([P, ncols], f32, tag="mlp_var")
                stt(var[:], mu[:], 1.0, mu[:], Alu.mult, Alu.mult)
                qn = sb.tile([P, ncols], f32, tag="mlp_qn")
                nc.vector.tensor_scalar_mul(out=qn[:], in0=Q[:],
                                            scalar1=1.0 / od)
                nc.vector.tensor_tensor(out=var[:], in0=qn[:], in1=var[:],
                                        op=Alu.subtract)
                sd = sb.tile([P, ncols], f32, tag="mlp_sd")
                nc.scalar.activation(out=sd[:], in_=var[:], func=Act.Sqrt,
                                     bias=cst[:, 4:5])
                rs = sb.tile([P, ncols], f32, tag="mlp_rs")
                nc.vector.reciprocal(out=rs[:], in_=sd[:])
                tmp = sb.tile([P, ncols], f32, tag="mlp_tmp")
                stt(tmp[:], mu[:], neg_s2_col, D_[:], Alu.mult, Alu.add)
                nc.vector.tensor_tensor(out=tmp[:], in0=tmp[:], in1=rs[:],
                                        op=Alu.mult)
                stt(out_tile[:], tmp[:], 1.0,
                    b2e_col.to_broadcast([P, ncols]), Alu.mult, Alu.add)

            fence()

            def stub_outputs():
                dumf = cpool.tile([P, fcols], f32, tag="dumf")
                nc.gpsimd.memset(dumf[:], 0.0)
                dume = cpool.tile([P, ecols], f32, tag="dume")
                nc.gpsimd.memset(dume[:], 0.0)
                DMA(out=r2(y_out), in_=dumf[:])
                DMA(out=r2(y2_out), in_=dume[:])

            # ============================================================
            # Phase A (+ phase-B stream prep overlapped)
            # ============================================================
            gc = t1 // cfg.gather_chunks
            with tc.tile_pool(name="bws", bufs=1) as bws, \
                    tc.tile_pool(name="bpool", bufs=3) as bpl:
                SQD_A = [cpool.tile([P, spr], f32, tag=f"sqd_a{i}",
                                     name=f"sqd_a{i}")
                         for i in range(3)]
                out_sb = cpool.tile([P, spr], f32, tag="out_sb")
                nmac = toka // cfg.a_mac
                msub = cfg.a_mac // P
                xTk = xT.rearrange("(k p) c -> p k c", k=kch)
                for mblk in range(nmac):
                    xt = sb.tile([P, kch * cfg.a_mac], bf16, tag="xt")
                    DMA(
                        out=xt[:],
                        in_=xTk[:, :,
                                mblk * cfg.a_mac:(mblk + 1) * cfg.a_mac])

                    def lhsT_a(s, _xt=xt):
                        return [(_xt[:, k * cfg.a_mac + s * P:
                                     k * cfg.a_mac + (s + 1) * P], w1_sb[k])
                                for k in range(kch)]

                    mlp_accum(msub, lhsT_a, *SQD_A, mblk * msub, w2b_sb)
                mlp_finish(*SQD_A, spr, cst[:, 0:1], cst[:, 1:2], out_sb)
                out_bb = cpool.tile([P, spr], bf16, tag="out_bb")
                nc.vector.tensor_copy(out=out_bb[:], in_=out_sb[:])
                DMA(out=r2(out_part), in_=out_bb[:])

                # -- phase-B stream prep (no out_tab dependency)
                bpT = bws.tile([P, t1], f32, tag="bpT")
                DMA(out=bpT[:], in_=bp[:, :])
                bmT = bws.tile([P, t1], f32, tag="bmT")
                DMA(out=bmT[:], in_=bmsk[:, :])
                bposT = bws.tile([P, t1], i32, tag="bposT")
                DMA(out=bposT[:], in_=bpos[:, :])
                wstage = bws.tile([P, t1], f32, tag="wstage")
                ustage = bws.tile([P, t1], f32, tag="ustage")
                stt(bpT[:], bpT[:], cfg.neg_slope, bpT[:], Alu.mult,
                    Alu.max)
                nc.scalar.activation(out=bpT[:], in_=bpT[:], func=Act.Exp)
                nc.vector.tensor_tensor_scan(
                    out=wstage[:], data0=bmT[:], data1=bpT[:],
                    initial=0.0, op0=Alu.mult, op1=Alu.add)
                DMA(out=r2(bredW), in_=wstage[:])

                CC(
                    "AllGather", Alu.bypass, replica_groups=groups,
                    ins=[out_part[:]], outs=[out_tab[0:n_cyc]])
                DMA(out=r2(out_tab[n_cyc:n_cyc + 16], p=1),
                    in_=sent0[:])
                fence()

                if "b" not in cfg.phases:
                    stub_outputs()
                    return

                # ========================================================
                # Phase B gathers + weighted scan
                # ========================================================
                for i in range(cfg.gather_chunks):
                    sl = slice(i * gc, (i + 1) * gc)
                    gct = bpl.tile([P, gc], bf16, tag="gct")
                    IDMA(
                        out=gct[:], out_offset=None, in_=col(out_tab[:]),
                        in_offset=bass.IndirectOffsetOnAxis(
                            ap=bposT[:, sl], axis=0))
                    gctf = bpl.tile([P, gc], f32, tag="gctf")
                    nc.vector.tensor_tensor(out=gctf[:], in0=bpT[:, sl],
                                            in1=gct[:], op=Alu.mult)
                    nc.vector.tensor_tensor_scan(
                        out=ustage[:, sl], data0=bmT[:, sl], data1=gctf[:],
                        initial=(0.0 if i == 0 else
                                 ustage[:, i * gc - 1:i * gc]),
                        op0=Alu.mult, op1=Alu.add)
                DMA(out=r2(bredU), in_=ustage[:])
                fence()

            dW = cpool.tile([P, bcols], f32, tag="dW")
            IDMA(
                out=dW[:], out_offset=None, in_=col(bredW[:]),
                in_offset=bass.IndirectOffsetOnAxis(ap=bidx_sb[:], axis=0))
            dU = cpool.tile([P, bcols], f32, tag="dU")
            IDMA(
                out=dU[:], out_offset=None, in_=col(bredU[:]),
                in_offset=bass.IndirectOffsetOnAxis(ap=bidx_sb[:], axis=0))
            o2 = cpool.tile([P, bcols], f32, tag="o2")
            nc.vector.tensor_scalar_add(out=o2[:], in0=dW[:], scalar1=1e-30)
            nc.vector.reciprocal(out=o2[:], in_=o2[:])
            nc.vector.tensor_tensor(out=o2[:], in0=o2[:], in1=dU[:],
                                    op=Alu.mult)
            o2b = cpool.tile([P, bcols], bf16, tag="o2b")
            nc.vector.tensor_copy(out=o2b[:], in_=o2[:])
            DMA(out=r2(out2_part), in_=o2b[:])

            # ============================================================
            # Phase C (stream loads overlap the out2 AllGather)
            # ============================================================
            gc2 = t2 // cfg.gather_chunks
            with tc.tile_pool(name="cws", bufs=1) as cws, \
                    tc.tile_pool(name="cpool2", bufs=3) as cpl:
                cmT = cws.tile([P, t2], f32, tag="cmT")
                DMA(out=cmT[:], in_=cmsk[:, :])
                csT = cws.tile([P, t2], i32, tag="csT")
                DMA(out=csT[:], in_=csrc[:, :])
                mstage = cws.tile([P, t2], f32, tag="mstage")
                fence()
                CC(
                    "AllGather", Alu.bypass, replica_groups=groups,
                    ins=[out2_part[:]], outs=[out2_tab[0:n_cyc]])
                DMA(out=r2(out2_tab[n_cyc:n_cyc + 16], p=1),
                    in_=sent1[:])
                fence()

                if "c" not in cfg.phases:
                    stub_outputs()
                    return

                for i in range(cfg.gather_chunks):
                    sl = slice(i * gc2, (i + 1) * gc2)
                    g2t = cpl.tile([P, gc2], bf16, tag="g2t")
                    IDMA(
                        out=g2t[:], out_offset=None, in_=col(out2_tab[:]),
                        in_offset=bass.IndirectOffsetOnAxis(
                            ap=csT[:, sl], axis=0))
                    nc.vector.tensor_tensor_scan(
                        out=mstage[:, sl], data0=cmT[:, sl], data1=g2t[:],
                        initial=(0.0 if i == 0 else
                                 mstage[:, i * gc2 - 1:i * gc2]),
                        op0=Alu.mult, op1=Alu.max)
                DMA(out=r2(credM), in_=mstage[:])
                fence()

            # local per-target maxima -> om -> featT row 0
            omr = cpool.tile([P, fcols], f32, tag="omr")
            IDMA(
                out=omr[:], out_offset=None, in_=col(credM[:]),
                in_offset=bass.IndirectOffsetOnAxis(ap=cidx_sb[:], axis=0))
            omm = cpool.tile([P, fcols], f32, tag="omm")
            nc.vector.tensor_scalar(out=omm[:], in0=omr[:], scalar1=-1e29,
                                    scalar2=None, op0=Alu.is_gt)
            om = cpool.tile([P, fcols], f32, tag="om")
            nc.vector.tensor_tensor(out=om[:], in0=omr[:], in1=omm[:],
                                    op=Alu.mult)
            DMA(out=r2(featT[0, :]), in_=om[:])
            fence()

            if "d" not in cfg.phases:
                stub_outputs()
                return

            # ============================================================
            # Phase D: full MLP on nonempty targets
            # ============================================================
            yF = cpool.tile([P, fcols], f32, tag="yF")
            SQD_D = [cpool.tile([P, fcols], f32, tag=f"sqd_d{i}",
                                 name=f"sqd_d{i}")
                     for i in range(3)]
            nchunk = tokdf // cfg.d_chunk
            dsub = cfg.d_chunk // P
            with tc.tile_pool(name="dpool", bufs=2) as dpl:
                for ch in range(nchunk):
                    ft = dpl.tile([2, cfg.d_chunk], f32, tag="ft")
                    DMA(
                        out=ft[:],
                        in_=featT[:, ch * cfg.d_chunk:(ch + 1) * cfg.d_chunk])

                    def lhsT_d(s, _ft=ft):
                        return [(_ft[:, s * P:(s + 1) * P], wk1_sb)]

                    mlp_accum(dsub, lhsT_d, *SQD_D, ch * dsub, wk2b_sb)
            mlp_finish(*SQD_D, fcols, cst[:, 2:3], cst[:, 3:4], yF)

            fence()

            # ---- fast path: empty targets (om == 0)
            wk2t = cpool.tile([P, ecols], f32, tag="wk2t")
            stt(wk2t[:], wke_sb[:], 1.0, wke_sb[:], Alu.mult, Alu.mult)
            nc.vector.tensor_scalar(out=wk2t[:], in0=wk2t[:],
                                    scalar1=cst[:, 5:6], scalar2=cst[:, 4:5],
                                    op0=Alu.mult, op1=Alu.add)
            nc.scalar.activation(out=wk2t[:], in_=wk2t[:], func=Act.Sqrt)
            nc.vector.reciprocal(out=wk2t[:], in_=wk2t[:])
            yE = cpool.tile([P, ecols], f32, tag="yE")
            nc.vector.tensor_scalar(out=yE[:], in0=wke_sb[:],
                                    scalar1=cst[:, 6:7], scalar2=None,
                                    op0=Alu.mult)
            nc.vector.tensor_tensor(out=yE[:], in0=yE[:], in1=wk2t[:],
                                    op=Alu.mult)
            nc.vector.tensor_scalar(out=yE[:], in0=yE[:], scalar1=cst[:, 3:4],
                                    scalar2=None, op0=Alu.add)

            # ---- global L2 norm
            ssq = cpool.tile([P, 2], f32, tag="ssq")
            scrF = cpool.tile([P, fcols], f32, tag="scrF")
            nc.vector.tensor_tensor(out=scrF[:], in0=yF[:], in1=mf_sb[:],
                                    op=Alu.mult)
            scrF2 = cpool.tile([P, fcols], f32, tag="scrF2")
            stt(scrF2[:], scrF[:], 1.0, yF[:], Alu.mult, Alu.mult,
                accum=ssq[:, 0:1])
            scrE = cpool.tile([P, ecols], f32, tag="scrE")
            nc.vector.tensor_tensor(out=scrE[:], in0=yE[:], in1=me_sb[:],
                                    op=Alu.mult)
            scrE2 = cpool.tile([P, ecols], f32, tag="scrE2")
            stt(scrE2[:], scrE[:], 1.0, yE[:], Alu.mult, Alu.mult,
                accum=ssq[:, 1:2])
            ssqt = cpool.tile([P, 1], f32, tag="ssqt")
            nc.vector.tensor_tensor(out=ssqt[:], in0=ssq[:, 0:1],
                                    in1=ssq[:, 1:2], op=Alu.add)
            ones = cpool.tile([P, 1], f32, tag="ones")
            nc.gpsimd.memset(ones[:], 1.0)
            sred = ps1.tile([1, 1], f32, tag="sred")
            nc.tensor.matmul(out=sred[:], lhsT=ones[:], rhs=ssqt[:],
                             start=True, stop=True)
            nsq_sb = cpool.tile([1, 16], f32, tag="nsq_sb")
            nc.gpsimd.memset(nsq_sb[:], 0.0)
            nc.vector.tensor_copy(out=nsq_sb[:, 0:1], in_=sred[:])
            DMA(out=r2(nsq_part, p=1), in_=nsq_sb[:])
            fence()
            CC(
                "AllReduce", Alu.add, replica_groups=groups,
                ins=[nsq_part[:]], outs=[nsq_tab[:]])
            fence()
            nrm = cpool.tile([1, 1], f32, tag="nrm")
            DMA(out=nrm[:], in_=r2(nsq_tab[0:1], p=1))
            nc.scalar.activation(out=nrm[:], in_=nrm[:], func=Act.Sqrt)
            nc.vector.tensor_scalar_max(out=nrm[:], in0=nrm[:],
                                        scalar1=1e-12)
            nc.vector.reciprocal(out=nrm[:], in_=nrm[:])
            ones_row = cpool.tile([1, P], f32, tag="ones_row")
            nc.gpsimd.memset(ones_row[:], 1.0)
            rn_ps = ps1.tile([P, 1], f32, tag="rn_ps")
            nc.tensor.matmul(out=rn_ps[:], lhsT=ones_row[:], rhs=nrm[:],
                             start=True, stop=True)
            rn_sb = cpool.tile([P, 1], f32, tag="rn_sb")
            nc.vector.tensor_copy(out=rn_sb[:], in_=rn_ps[:])

            # sigmoid(x) = 1/(1+exp(-x)) via Exp + HW reciprocal: the ACT
            # sigmoid table's absolute error lands directly on the output.
            nrn_sb = cpool.tile([P, 1], f32, tag="nrn_sb")
            nc.vector.tensor_scalar_mul(out=nrn_sb[:], in0=rn_sb[:],
                                        scalar1=-1.0)

            def scale_sigmoid(t):
                nc.scalar.activation(out=t, in_=t, func=Act.Exp,
                                     scale=nrn_sb[:, 0:1])
                nc.vector.tensor_scalar_add(out=t, in0=t, scalar1=1.0)
                nc.vector.reciprocal(out=t, in_=t)

            scale_sigmoid(yF[:])
            DMA(out=r2(y_out), in_=yF[:])
            scale_sigmoid(yE[:])
            DMA(out=r2(y2_out), in_=yE[:])
            fence()

        for _rep in range(cfg.reps):
            emit_body()

    return nc


# ---------------------------------------------------------------------------
# entry point
# ---------------------------------------------------------------------------

_NC_CACHE = {}


def _get_nc(cfg):
    key = (cfg.n_cyc, cfg.e_cc, cfg.len_edges, cfg.t1, cfg.t2,
           cfg.tokd_full, cfg.reps, cfg.phases)
    if key not in _NC_CACHE:
        nc = build_nc(cfg)
        if not nc.is_finalized():
            nc.finalize()
        _NC_CACHE[key] = nc
    return _NC_CACHE[key]


def run(inputs, cfg=None, trace=False):
    from concourse.bass_utils import run_bass_kernel_spmd
    cfg = cfg or Cfg()
    in_maps, asm = host_prepare(inputs, cfg)
    nc = _get_nc(cfg)
    res = run_bass_kernel_spmd(nc, in_maps, core_ids=list(range(NCORES)),
                               trace=trace)
    return assemble_output(res.results, asm, cfg), res


def kernel(**inputs):
    out, _ = run(inputs)
    return out
